# revision 8
# baseline (speedup 1.0000x reference)
"""TRN2 Bass kernel for nn_ExpertTimmViTBlock (B=8, N=1024, C=1024, H=16).

Sharding: data-parallel over batch, one batch element per NeuronCore (8 cores).
Each core runs the full ViT block on its [N, C] slice; no collectives.

Per-core dataflow (activations kept feature-major for matmuls, token-major
for layernorms; fp32r matmuls everywhere for full-rate fp32-ish precision):

  x [tok,C] --PE-transpose--> Xc = x^T (feature-major, f32r)
  v = x @ Wv (lhsT = Xc chunks) -> v' token-major [tok, H, 65] (col 64 = ones)
  per head pair: q^T, k^T = Wqk^T @ x^T (rotating tiles), then
     S^T = k q^T (row-packed pairs), E = exp(S^T*scale)
     y_un^T/denom = v'^T @ E^T  (M=65, denom lands in row 64)
     broadcast 1/denom via K=1 matmul at row 64, normalize -> ycat
  proj (token-major out) + bias; y2 = attn + LN1(attn); PE-transpose -> y2T
  h^T = gelu(W1^T y2^T + b) (feature-major, 512-token slabs)
  h2^T = W2^T h^T + b -> PE-transpose -> h2 token-major
  out = y2 + LN2(h2)

SBUF pressure is managed by slot reuse: ycat slots are rewritten as y2T,
v' slots as y2 (the Tile framework serializes via WAR deps).
"""
import sys

if '/opt/trn_rl_repo' not in sys.path:
    sys.path.insert(0, '/opt/trn_rl_repo')

import numpy as np
import concourse.bass as bass
import concourse.tile as tile
from concourse import bacc, mybir
from concourse.bass_utils import run_bass_kernel_spmd
from concourse.masks import make_identity

F32 = mybir.dt.float32
F32R = mybir.dt.float32r
AF = mybir.ActivationFunctionType
ALU = mybir.AluOpType

B, N, C, H = 8, 1024, 1024, 16
DH = C // H          # 64
C3, C4 = 3 * C, 4 * C
SCALE = DH ** -0.5
EPS = 1e-6
TT = N // 128        # 8 token tiles
CC = C // 128        # 8 feature chunks
HC = C4 // 128       # 32 hidden chunks
QT = N // 512        # 2 query (token) slabs of 512
NPAIR = H // 2       # 8 head pairs


def _ln_apply(nc, pool, a, g_bc, b_bc, eps_t, out, residual):
    """out = residual + layernorm(a)*g + b   (token-major [128, C] tiles)."""
    stats = pool.tile([128, 2, 6], F32, tag="ln_st")
    nc.vector.bn_stats(stats[:, 0, :], a[:, 0:512])
    nc.vector.bn_stats(stats[:, 1, :], a[:, 512:1024])
    mv = pool.tile([128, 2], F32, tag="ln_mv")
    nc.vector.bn_aggr(mv, stats)
    std = pool.tile([128, 1], F32, tag="ln_sd")
    nc.scalar.activation(std, mv[:, 1:2], AF.Sqrt, bias=eps_t)
    rstd = pool.tile([128, 1], F32, tag="ln_rs")
    nc.vector.reciprocal(rstd, std)
    t1 = pool.tile([128, C], F32, tag="ln_t1", bufs=1)
    nc.vector.tensor_scalar(t1, a, scalar1=mv[:, 0:1], scalar2=rstd,
                            op0=ALU.subtract, op1=ALU.mult)
    nc.vector.tensor_tensor(t1, t1, g_bc, op=ALU.mult)
    nc.vector.tensor_tensor(t1, t1, b_bc, op=ALU.add)
    nc.vector.tensor_tensor(out, t1, residual, op=ALU.add)


def build():
    nc = bacc.Bacc("TRN2", target_bir_lowering=False, debug=False)

    x = nc.dram_tensor("x", [N, C], F32, kind="ExternalInput").ap()
    qkv_w = nc.dram_tensor("qkv_w", [C, C3], F32R, kind="ExternalInput").ap()
    qkv_b = nc.dram_tensor("qkv_b", [C3], F32, kind="ExternalInput").ap()
    proj_w = nc.dram_tensor("proj_w", [C, C], F32R, kind="ExternalInput").ap()
    proj_b = nc.dram_tensor("proj_b", [C], F32, kind="ExternalInput").ap()
    n1_g = nc.dram_tensor("n1_g", [C], F32, kind="ExternalInput").ap()
    n1_b = nc.dram_tensor("n1_b", [C], F32, kind="ExternalInput").ap()
    fc1_w = nc.dram_tensor("fc1_w", [C, C4], F32R, kind="ExternalInput").ap()
    fc1_b = nc.dram_tensor("fc1_b", [C4], F32, kind="ExternalInput").ap()
    fc2_w = nc.dram_tensor("fc2_w", [C4, C], F32R, kind="ExternalInput").ap()
    fc2_b = nc.dram_tensor("fc2_b", [C], F32, kind="ExternalInput").ap()
    n2_g = nc.dram_tensor("n2_g", [C], F32, kind="ExternalInput").ap()
    n2_b = nc.dram_tensor("n2_b", [C], F32, kind="ExternalInput").ap()
    out = nc.dram_tensor("out", [N, C], F32, kind="ExternalOutput").ap()

    ones_dram = nc.inline_tensor(np.ones((128, 64), np.float32), name="onesc")

    with tile.TileContext(nc) as tc:
        with tc.tile_pool(name="consts", bufs=1) as consts, \
             tc.tile_pool(name="lnp", bufs=2) as lnp, \
             tc.tile_pool(name="vp", bufs=1) as vp, \
             tc.tile_pool(name="ycp", bufs=1) as ycp:
            consts_e_cm = tc.tile_pool(name="consts_e", bufs=1)
            consts_e = consts_e_cm.__enter__()
            xcp_cm = tc.tile_pool(name="xcp", bufs=1)
            xcp = xcp_cm.__enter__()
            qkp_cm = tc.tile_pool(name="qkp", bufs=1)
            qkp = qkp_cm.__enter__()

            # ---------------- constants ----------------
            idt = consts.tile([128, 128], F32)
            make_identity(nc, idt)
            onesc = consts.tile([128, 64], F32R)
            nc.sync.dma_start(onesc, ones_dram.ap().bitcast(F32R))
            eps_t = consts.tile([128, 1], F32)
            nc.vector.memset(eps_t, EPS)
            qkb = consts.tile([128, 16], F32)
            nc.sync.dma_start(qkb, qkv_b[0:2048].rearrange("(c p) -> p c", p=128))
            fc1b = consts.tile([128, HC], F32)
            nc.sync.dma_start(fc1b, fc1_b.rearrange("(c p) -> p c", p=128))
            fc2b = consts.tile([128, CC], F32)
            nc.sync.dma_start(fc2b, fc2_b.rearrange("(c p) -> p c", p=128))
            n2g_bc = consts.tile([128, C], F32)
            nc.sync.dma_start(n2g_bc, n2_g.partition_broadcast(128))
            n2b_bc = consts.tile([128, C], F32)
            nc.sync.dma_start(n2b_bc, n2_b.partition_broadcast(128))

            vb_bc = consts_e.tile([128, C], F32)
            nc.sync.dma_start(vb_bc, qkv_b[2048:3072].partition_broadcast(128))
            pb_bc = consts_e.tile([128, C], F32)
            nc.sync.dma_start(pb_bc, proj_b.partition_broadcast(128))
            n1g_bc = consts_e.tile([128, C], F32)
            nc.sync.dma_start(n1g_bc, n1_g.partition_broadcast(128))
            n1b_bc = consts_e.tile([128, C], F32)
            nc.sync.dma_start(n1b_bc, n1_b.partition_broadcast(128))

            xc = [xcp.tile([128, N], F32R, tag=f"xc{c}", name=f"xc{c}")
                  for c in range(CC)]
            vtk = [vp.tile([128, H, DH + 1], F32R, tag=f"v{t}", name=f"v{t}")
                   for t in range(TT)]
            ycat = [ycp.tile([128, N], F32R, tag=f"yc{p}", name=f"yc{p}")
                    for p in range(NPAIR)]

            # ---------------- phase 1: transpose x -> Xc ----------------
            with tc.tile_pool(name="xin", bufs=2) as xin, \
                 tc.tile_pool(name="tpx", bufs=4, space="PSUM") as tpx:
                for t in range(TT):
                    xt = xin.tile([128, C], F32, tag="x")
                    nc.sync.dma_start(xt, x[t * 128:(t + 1) * 128, :])
                    for c in range(CC):
                        ps = tpx.tile([128, 128], F32, tag="t")
                        nc.tensor.transpose(ps, xt[:, bass.ts(c, 128)], idt)
                        nc.vector.tensor_copy(xc[c][:, bass.ts(t, 128)], ps)

            # ---------------- phase 2a: v token-major ----------------
            for t in range(TT):
                nc.sync.dma_start(
                    vtk[t][:, :, DH:DH + 1],
                    ones_dram.ap().bitcast(F32R)[:, 0:H].rearrange(
                        "p (h o) -> p h o", o=1))
            with tc.tile_pool(name="wv", bufs=3) as wvp, \
                 tc.tile_pool(name="ppv", bufs=1, space="PSUM") as ppv:
                for ts_ in range(2):          # token-tile slabs of 4
                    for vt in range(2):       # v feature halves
                        pvs = [ppv.tile([128, 512], F32, tag=f"pv{i}", name=f"pv{i}")
                               for i in range(4)]
                        for c in range(CC):
                            wv = wvp.tile([128, 512], F32R, tag="wv")
                            nc.sync.dma_start(
                                wv, qkv_w[c * 128:(c + 1) * 128,
                                          2048 + vt * 512: 2048 + (vt + 1) * 512])
                            for i in range(4):
                                t = ts_ * 4 + i
                                nc.tensor.matmul(pvs[i], xc[c][:, bass.ts(t, 128)],
                                                 wv, start=(c == 0), stop=(c == CC - 1))
                        for i in range(4):
                            t = ts_ * 4 + i
                            nc.vector.tensor_tensor(
                                vtk[t][:, vt * 8:(vt + 1) * 8, 0:DH],
                                pvs[i].rearrange("p (h d) -> p h d", d=DH),
                                vb_bc[:, vt * 512:(vt + 1) * 512].rearrange(
                                    "p (h d) -> p h d", d=DH),
                                op=ALU.add)

            # ------- phase 2b: per pair: q^T,k^T production + attention -------
            with tc.tile_pool(name="wqk", bufs=6) as wqk, \
                 tc.tile_pool(name="ep", bufs=2) as ep, \
                 tc.tile_pool(name="nrm", bufs=2) as nrm, \
                 tc.tile_pool(name="pqs", bufs=1, space="PSUM") as pqs, \
                 tc.tile_pool(name="psc", bufs=1, space="PSUM") as psc, \
                 tc.tile_pool(name="py", bufs=2, space="PSUM") as py:
                for p in range(NPAIR):
                    qk_tiles = []
                    for j, oc in enumerate((p, 8 + p)):       # q then k
                        dst = qkp.tile([128, N], F32R, tag=f"qk{j}_{p % 2}",
                                       name=f"qk{j}_{p}")
                        for half in range(2):
                            pqk = pqs.tile([128, 512], F32, tag=f"q{half}",
                                           name=f"pq{half}")
                            for c in range(CC):
                                w = wqk.tile([128, 128], F32R, tag="w")
                                nc.sync.dma_start(
                                    w, qkv_w[c * 128:(c + 1) * 128,
                                             oc * 128:(oc + 1) * 128])
                                nc.tensor.matmul(pqk, w, xc[c][:, bass.ts(half, 512)],
                                                 start=(c == 0), stop=(c == CC - 1))
                            nc.scalar.activation(dst[:, bass.ts(half, 512)], pqk,
                                                 AF.Identity, bias=qkb[:, oc:oc + 1])
                        qk_tiles.append(dst)
                    qT, kT = qk_tiles

                    for qt in range(QT):
                        qsl = bass.ts(qt, 512)
                        yps = [py.tile([65, 512], F32, tag=f"yp{i}", name=f"yp{i}")
                               for i in range(2)]
                        for kt in range(TT):
                            es = []
                            for i, r0 in enumerate((0, 64)):
                                s = psc.tile([128, 512], F32, tag=f"sp{i}",
                                             name=f"sp{i}")
                                nc.tensor.matmul(
                                    s, kT[r0:r0 + 64, bass.ts(kt, 128)],
                                    qT[r0:r0 + 64, qsl], start=True, stop=True)
                                e = ep.tile([128, 512], F32R, tag=f"e{i}",
                                            name=f"e{i}")
                                nc.scalar.activation(e, s, AF.Exp, scale=SCALE)
                                es.append(e)
                            for i in range(2):
                                nc.tensor.matmul(yps[i], vtk[kt][:, 2 * p + i, :],
                                                 es[i], start=(kt == 0),
                                                 stop=(kt == TT - 1))
                        for i in range(2):
                            dsb = nrm.tile([65, 512], F32R, tag=f"dsb{i}",
                                           name=f"dsb{i}")
                            nc.vector.tensor_copy(dsb[64:65, :], yps[i][64:65, :])
                            bp = psc.tile([64, 512], F32, tag="sp0", name="bp")
                            nc.tensor.matmul(bp, onesc[64:65, 0:64], dsb[64:65, :],
                                             start=True, stop=True)
                            rc = nrm.tile([64, 512], F32, tag=f"rc{i}", name=f"rc{i}")
                            nc.vector.reciprocal(rc, bp)
                            if i == 0:
                                nc.vector.tensor_tensor(ycat[p][0:64, qsl],
                                                        yps[i][0:64, :], rc,
                                                        op=ALU.mult)
                            else:
                                yt = nrm.tile([64, 512], F32R, tag="yt")
                                nc.vector.tensor_tensor(yt, yps[i][0:64, :], rc,
                                                        op=ALU.mult)
                                nc.sync.dma_start(ycat[p][64:128, qsl], yt)

            qkp_cm.__exit__(None, None, None)
            xcp_cm.__exit__(None, None, None)
            # ---------------- phase 4: proj + LN1 + transpose ----------------
            y2 = [vp.tile([128, C], F32, tag=f"v{t}", name=f"y2_{t}")
                  for t in range(TT)]
            y2T = [ycp.tile([128, N], F32R, tag=f"yc{c}", name=f"y2T{c}")
                   for c in range(CC)]
            with tc.tile_pool(name="wpj", bufs=1) as wpj, \
                 tc.tile_pool(name="atn", bufs=2) as atn, \
                 tc.tile_pool(name="ppj", bufs=2, space="PSUM") as ppj, \
                 tc.tile_pool(name="tpy", bufs=4, space="PSUM") as tpy:
                wp = [wpj.tile([128, C], F32R, tag=f"wp{c}", name=f"wp{c}")
                      for c in range(CC)]
                for c in range(CC):
                    nc.sync.dma_start(wp[c], proj_w[c * 128:(c + 1) * 128, :])
                for t in range(TT):
                    ps0 = ppj.tile([128, 512], F32, tag="a")
                    ps1 = ppj.tile([128, 512], F32, tag="b")
                    for c in range(CC):
                        st, sp = (c == 0), (c == CC - 1)
                        nc.tensor.matmul(ps0, ycat[c][:, bass.ts(t, 128)],
                                         wp[c][:, 0:512], start=st, stop=sp)
                        nc.tensor.matmul(ps1, ycat[c][:, bass.ts(t, 128)],
                                         wp[c][:, 512:1024], start=st, stop=sp)
                    at = atn.tile([128, C], F32, tag="at")
                    nc.vector.tensor_tensor(at[:, 0:512], ps0, pb_bc[:, 0:512],
                                            op=ALU.add)
                    nc.vector.tensor_tensor(at[:, 512:1024], ps1, pb_bc[:, 512:1024],
                                            op=ALU.add)
                    _ln_apply(nc, lnp, at, n1g_bc, n1b_bc, eps_t, y2[t], at)
                    for c in range(CC):
                        ps = tpy.tile([128, 128], F32, tag="t")
                        nc.tensor.transpose(ps, y2[t][:, bass.ts(c, 128)], idt)
                        nc.vector.tensor_copy(y2T[c][:, bass.ts(t, 128)], ps)

            consts_e_cm.__exit__(None, None, None)
            # ---------------- phase 5-7: MLP + LN2 + out (512-token slabs) ----
            with tc.tile_pool(name="hTp", bufs=1) as hTp, \
                 tc.tile_pool(name="w12", bufs=6) as w12, \
                 tc.tile_pool(name="h2t", bufs=3) as h2t, \
                 tc.tile_pool(name="h2p", bufs=1) as h2p, \
                 tc.tile_pool(name="fin", bufs=2) as fin, \
                 tc.tile_pool(name="pf1", bufs=3, space="PSUM") as pf1, \
                 tc.tile_pool(name="pf2", bufs=3, space="PSUM") as pf2, \
                 tc.tile_pool(name="tph", bufs=2, space="PSUM") as tph:
                for s in range(QT):
                    ssl = bass.ts(s, 512)
                    hT = [hTp.tile([128, 512], F32R, tag=f"h{hc}", name=f"h{hc}")
                          for hc in range(HC)]
                    h2 = [h2p.tile([128, C], F32, tag=f"h2_{i}", name=f"h2_{s}_{i}")
                          for i in range(4)]
                    # fc1 + gelu -> h^T (feature-major)
                    for hc in range(HC):
                        ph = pf1.tile([128, 512], F32, tag="a")
                        for c in range(CC):
                            w1 = w12.tile([128, 128], F32R, tag="w1")
                            nc.sync.dma_start(
                                w1, fc1_w[c * 128:(c + 1) * 128,
                                          hc * 128:(hc + 1) * 128])
                            nc.tensor.matmul(ph, w1, y2T[c][:, ssl],
                                             start=(c == 0), stop=(c == CC - 1))
                        nc.scalar.activation(hT[hc], ph, AF.Gelu,
                                             bias=fc1b[:, hc:hc + 1])
                    # fc2 -> h2^T chunks, transpose to token-major h2
                    for ct in range(CC):
                        pq = pf2.tile([128, 512], F32, tag="a")
                        for hc in range(HC):
                            w2 = w12.tile([128, 128], F32R, tag="w2")
                            nc.sync.dma_start(
                                w2, fc2_w[hc * 128:(hc + 1) * 128,
                                          ct * 128:(ct + 1) * 128])
                            nc.tensor.matmul(pq, w2, hT[hc],
                                             start=(hc == 0), stop=(hc == HC - 1))
                        h2T = h2t.tile([128, 512], F32, tag="h2T")
                        nc.scalar.activation(h2T, pq, AF.Identity,
                                             bias=fc2b[:, ct:ct + 1])
                        for i in range(4):
                            ps = tph.tile([128, 128], F32, tag="t")
                            nc.tensor.transpose(ps, h2T[:, bass.ts(i, 128)], idt)
                            nc.vector.tensor_copy(h2[i][:, bass.ts(ct, 128)], ps)
                    # LN2 + residual + store
                    for i in range(4):
                        t = s * 4 + i
                        ot = fin.tile([128, C], F32, tag="o")
                        _ln_apply(nc, lnp, h2[i], n2g_bc, n2b_bc, eps_t, ot, y2[t])
                        nc.sync.dma_start(out[t * 128:(t + 1) * 128, :], ot)

    nc.compile()
    return nc


_NC_CACHE = None


def kernel(**inputs):
    global _NC_CACHE
    if _NC_CACHE is None:
        _NC_CACHE = build()
    nc = _NC_CACHE

    wnames = ["qkv_w", "qkv_b", "proj_w", "proj_b", "n1_g", "n1_b",
              "fc1_w", "fc1_b", "fc2_w", "fc2_b", "n2_g", "n2_b"]
    shared = {k: np.ascontiguousarray(np.asarray(inputs[k], dtype=np.float32))
              for k in wnames}
    x = np.asarray(inputs["x"], dtype=np.float32)
    in_maps = [dict(shared, x=np.ascontiguousarray(x[b])) for b in range(B)]
    res = run_bass_kernel_spmd(nc, in_maps, list(range(B)))
    return np.stack([res.results[b]["out"] for b in range(B)]).astype(np.float32)


# revision 9
# speedup vs baseline: 173.6410x; 173.6410x over previous
"""TRN2 Bass kernel for nn_ExpertTimmViTBlock (B=8, N=1024, C=1024, H=16).

Sharding: data-parallel over batch, one batch element per NeuronCore (8 cores).
Each core runs the full ViT block on its [N, C] slice; no collectives.

Per-core dataflow (activations kept feature-major for matmuls, token-major
for layernorms; fp32r matmuls everywhere for full-rate fp32-ish precision):

  x [tok,C] --PE-transpose--> Xc = x^T (feature-major, f32r)
  v = x @ Wv (lhsT = Xc chunks) -> v' token-major [tok, H, 65] (col 64 = ones)
  per head pair: q^T, k^T = Wqk^T @ x^T (rotating tiles), then
     S^T = k q^T (row-packed pairs), E = exp(S^T*scale)
     y_un^T/denom = v'^T @ E^T  (M=65, denom lands in row 64)
     broadcast 1/denom via K=1 matmul at row 64, normalize -> ycat
  proj (token-major out) + bias; y2 = attn + LN1(attn); PE-transpose -> y2T
  h^T = gelu(W1^T y2^T + b) (feature-major, 512-token slabs)
  h2^T = W2^T h^T + b -> PE-transpose -> h2 token-major
  out = y2 + LN2(h2)

SBUF pressure is managed by slot reuse: ycat slots are rewritten as y2T,
v' slots as y2 (the Tile framework serializes via WAR deps).
"""
import sys

if '/opt/trn_rl_repo' not in sys.path:
    sys.path.insert(0, '/opt/trn_rl_repo')

import numpy as np
import concourse.bass as bass
import concourse.tile as tile
from concourse import bacc, mybir
from concourse.bass_utils import run_bass_kernel_spmd
from concourse.masks import make_identity

F32 = mybir.dt.float32
F32R = mybir.dt.float32r
AF = mybir.ActivationFunctionType
ALU = mybir.AluOpType

B, N, C, H = 8, 1024, 1024, 16
DH = C // H          # 64
C3, C4 = 3 * C, 4 * C
SCALE = DH ** -0.5
EPS = 1e-6
TT = N // 128        # 8 token tiles
CC = C // 128        # 8 feature chunks
HC = C4 // 128       # 32 hidden chunks
QT = N // 512        # 2 query (token) slabs of 512
NPAIR = H // 2       # 8 head pairs


def _ln_apply(nc, pool, a, g_bc, b_bc, eps_t, out, residual):
    """out = residual + layernorm(a)*g + b   (token-major [128, C] tiles)."""
    stats = pool.tile([128, 2, 6], F32, tag="ln_st")
    nc.vector.bn_stats(stats[:, 0, :], a[:, 0:512])
    nc.vector.bn_stats(stats[:, 1, :], a[:, 512:1024])
    mv = pool.tile([128, 2], F32, tag="ln_mv")
    nc.vector.bn_aggr(mv, stats)
    std = pool.tile([128, 1], F32, tag="ln_sd")
    nc.scalar.activation(std, mv[:, 1:2], AF.Sqrt, bias=eps_t)
    rstd = pool.tile([128, 1], F32, tag="ln_rs")
    nc.vector.reciprocal(rstd, std)
    t1 = pool.tile([128, C], F32, tag="ln_t1", bufs=1)
    nc.vector.tensor_scalar(t1, a, scalar1=mv[:, 0:1], scalar2=rstd,
                            op0=ALU.subtract, op1=ALU.mult)
    nc.vector.tensor_tensor(t1, t1, g_bc, op=ALU.mult)
    nc.vector.tensor_tensor(t1, t1, b_bc, op=ALU.add)
    nc.vector.tensor_tensor(out, t1, residual, op=ALU.add)


def build(repeat=1):
    nc = bacc.Bacc("TRN2", target_bir_lowering=False, debug=False)

    x = nc.dram_tensor("x", [N, C], F32, kind="ExternalInput").ap()
    qkv_w = nc.dram_tensor("qkv_w", [C, C3], F32R, kind="ExternalInput").ap()
    qkv_b = nc.dram_tensor("qkv_b", [C3], F32, kind="ExternalInput").ap()
    proj_w = nc.dram_tensor("proj_w", [C, C], F32R, kind="ExternalInput").ap()
    proj_b = nc.dram_tensor("proj_b", [C], F32, kind="ExternalInput").ap()
    n1_g = nc.dram_tensor("n1_g", [C], F32, kind="ExternalInput").ap()
    n1_b = nc.dram_tensor("n1_b", [C], F32, kind="ExternalInput").ap()
    fc1_w = nc.dram_tensor("fc1_w", [C, C4], F32R, kind="ExternalInput").ap()
    fc1_b = nc.dram_tensor("fc1_b", [C4], F32, kind="ExternalInput").ap()
    fc2_w = nc.dram_tensor("fc2_w", [C4, C], F32R, kind="ExternalInput").ap()
    fc2_b = nc.dram_tensor("fc2_b", [C], F32, kind="ExternalInput").ap()
    n2_g = nc.dram_tensor("n2_g", [C], F32, kind="ExternalInput").ap()
    n2_b = nc.dram_tensor("n2_b", [C], F32, kind="ExternalInput").ap()
    out = nc.dram_tensor("out", [N, C], F32, kind="ExternalOutput").ap()

    ones_dram = nc.inline_tensor(np.ones((128, 64), np.float32), name="onesc")

    with tile.TileContext(nc) as tc:
      for _rep in range(repeat):
        with tc.tile_pool(name="consts", bufs=1) as consts, \
             tc.tile_pool(name="lnp", bufs=2) as lnp, \
             tc.tile_pool(name="vp", bufs=1) as vp, \
             tc.tile_pool(name="ycp", bufs=1) as ycp:
            consts_e_cm = tc.tile_pool(name="consts_e", bufs=1)
            consts_e = consts_e_cm.__enter__()
            xcp_cm = tc.tile_pool(name="xcp", bufs=1)
            xcp = xcp_cm.__enter__()
            qkp_cm = tc.tile_pool(name="qkp", bufs=1)
            qkp = qkp_cm.__enter__()

            # ---------------- constants ----------------
            idt = consts.tile([128, 128], F32)
            make_identity(nc, idt)
            onesc = consts.tile([128, 64], F32R)
            nc.sync.dma_start(onesc, ones_dram.ap().bitcast(F32R))
            eps_t = consts.tile([128, 1], F32)
            nc.vector.memset(eps_t, EPS)
            qkb = consts.tile([128, 16], F32)
            nc.sync.dma_start(qkb, qkv_b[0:2048].rearrange("(c p) -> p c", p=128))
            fc1b = consts.tile([128, HC], F32)
            nc.sync.dma_start(fc1b, fc1_b.rearrange("(c p) -> p c", p=128))
            fc2b = consts.tile([128, CC], F32)
            nc.sync.dma_start(fc2b, fc2_b.rearrange("(c p) -> p c", p=128))
            n2g_bc = consts.tile([128, C], F32)
            nc.sync.dma_start(n2g_bc, n2_g.partition_broadcast(128))
            n2b_bc = consts.tile([128, C], F32)
            nc.sync.dma_start(n2b_bc, n2_b.partition_broadcast(128))

            vb_bc = consts_e.tile([128, C], F32)
            nc.sync.dma_start(vb_bc, qkv_b[2048:3072].partition_broadcast(128))
            pb_bc = consts_e.tile([128, C], F32)
            nc.sync.dma_start(pb_bc, proj_b.partition_broadcast(128))
            n1g_bc = consts_e.tile([128, C], F32)
            nc.sync.dma_start(n1g_bc, n1_g.partition_broadcast(128))
            n1b_bc = consts_e.tile([128, C], F32)
            nc.sync.dma_start(n1b_bc, n1_b.partition_broadcast(128))

            xc = [xcp.tile([128, N], F32R, tag=f"xc{c}", name=f"xc{c}")
                  for c in range(CC)]
            vtk = [vp.tile([128, H, DH + 1], F32R, tag=f"v{t}", name=f"v{t}")
                   for t in range(TT)]
            ycat = [ycp.tile([128, N], F32R, tag=f"yc{p}", name=f"yc{p}")
                    for p in range(NPAIR)]

            # ---------------- phase 1: transpose x -> Xc ----------------
            with tc.tile_pool(name="xin", bufs=2) as xin, \
                 tc.tile_pool(name="tpx", bufs=4, space="PSUM") as tpx:
                for t in range(TT):
                    xt = xin.tile([128, C], F32, tag="x")
                    nc.sync.dma_start(xt, x[t * 128:(t + 1) * 128, :])
                    for c in range(CC):
                        ps = tpx.tile([128, 128], F32, tag="t")
                        nc.tensor.transpose(ps, xt[:, bass.ts(c, 128)], idt)
                        nc.vector.tensor_copy(xc[c][:, bass.ts(t, 128)], ps)

            # ---------------- phase 2a: v token-major ----------------
            for t in range(TT):
                nc.sync.dma_start(
                    vtk[t][:, :, DH:DH + 1],
                    ones_dram.ap().bitcast(F32R)[:, 0:H].rearrange(
                        "p (h o) -> p h o", o=1))
            with tc.tile_pool(name="wv", bufs=3) as wvp, \
                 tc.tile_pool(name="ppv", bufs=1, space="PSUM") as ppv:
                for ts_ in range(2):          # token-tile slabs of 4
                    for vt in range(2):       # v feature halves
                        pvs = [ppv.tile([128, 512], F32, tag=f"pv{i}", name=f"pv{i}")
                               for i in range(4)]
                        for c in range(CC):
                            wv = wvp.tile([128, 512], F32R, tag="wv")
                            nc.sync.dma_start(
                                wv, qkv_w[c * 128:(c + 1) * 128,
                                          2048 + vt * 512: 2048 + (vt + 1) * 512])
                            for i in range(4):
                                t = ts_ * 4 + i
                                nc.tensor.matmul(pvs[i], xc[c][:, bass.ts(t, 128)],
                                                 wv, start=(c == 0), stop=(c == CC - 1))
                        for i in range(4):
                            t = ts_ * 4 + i
                            nc.vector.tensor_tensor(
                                vtk[t][:, vt * 8:(vt + 1) * 8, 0:DH],
                                pvs[i].rearrange("p (h d) -> p h d", d=DH),
                                vb_bc[:, vt * 512:(vt + 1) * 512].rearrange(
                                    "p (h d) -> p h d", d=DH),
                                op=ALU.add)

            # ------- phase 2b: per pair: q^T,k^T production + attention -------
            with tc.tile_pool(name="wqk", bufs=6) as wqk, \
                 tc.tile_pool(name="ep", bufs=2) as ep, \
                 tc.tile_pool(name="nrm", bufs=2) as nrm, \
                 tc.tile_pool(name="pqs", bufs=1, space="PSUM") as pqs, \
                 tc.tile_pool(name="psc", bufs=1, space="PSUM") as psc, \
                 tc.tile_pool(name="py", bufs=2, space="PSUM") as py:
                for p in range(NPAIR):
                    qk_tiles = []
                    for j, oc in enumerate((p, 8 + p)):       # q then k
                        dst = qkp.tile([128, N], F32R, tag=f"qk{j}_{p % 2}",
                                       name=f"qk{j}_{p}")
                        for half in range(2):
                            pqk = pqs.tile([128, 512], F32, tag=f"q{half}",
                                           name=f"pq{half}")
                            for c in range(CC):
                                w = wqk.tile([128, 128], F32R, tag="w")
                                nc.sync.dma_start(
                                    w, qkv_w[c * 128:(c + 1) * 128,
                                             oc * 128:(oc + 1) * 128])
                                nc.tensor.matmul(pqk, w, xc[c][:, bass.ts(half, 512)],
                                                 start=(c == 0), stop=(c == CC - 1))
                            nc.scalar.activation(dst[:, bass.ts(half, 512)], pqk,
                                                 AF.Identity, bias=qkb[:, oc:oc + 1])
                        qk_tiles.append(dst)
                    qT, kT = qk_tiles

                    for qt in range(QT):
                        qsl = bass.ts(qt, 512)
                        yps = [py.tile([65, 512], F32, tag=f"yp{i}", name=f"yp{i}")
                               for i in range(2)]
                        for kt in range(TT):
                            es = []
                            for i, r0 in enumerate((0, 64)):
                                s = psc.tile([128, 512], F32, tag=f"sp{i}",
                                             name=f"sp{i}")
                                nc.tensor.matmul(
                                    s, kT[r0:r0 + 64, bass.ts(kt, 128)],
                                    qT[r0:r0 + 64, qsl], start=True, stop=True)
                                e = ep.tile([128, 512], F32R, tag=f"e{i}",
                                            name=f"e{i}")
                                nc.scalar.activation(e, s, AF.Exp, scale=SCALE)
                                es.append(e)
                            for i in range(2):
                                nc.tensor.matmul(yps[i], vtk[kt][:, 2 * p + i, :],
                                                 es[i], start=(kt == 0),
                                                 stop=(kt == TT - 1))
                        for i in range(2):
                            dsb = nrm.tile([65, 512], F32R, tag=f"dsb{i}",
                                           name=f"dsb{i}")
                            nc.vector.tensor_copy(dsb[64:65, :], yps[i][64:65, :])
                            bp = psc.tile([64, 512], F32, tag="sp0", name="bp")
                            nc.tensor.matmul(bp, onesc[64:65, 0:64], dsb[64:65, :],
                                             start=True, stop=True)
                            rc = nrm.tile([64, 512], F32, tag=f"rc{i}", name=f"rc{i}")
                            nc.vector.reciprocal(rc, bp)
                            if i == 0:
                                nc.vector.tensor_tensor(ycat[p][0:64, qsl],
                                                        yps[i][0:64, :], rc,
                                                        op=ALU.mult)
                            else:
                                yt = nrm.tile([64, 512], F32R, tag="yt")
                                nc.vector.tensor_tensor(yt, yps[i][0:64, :], rc,
                                                        op=ALU.mult)
                                nc.sync.dma_start(ycat[p][64:128, qsl], yt)

            qkp_cm.__exit__(None, None, None)
            xcp_cm.__exit__(None, None, None)
            # ---------------- phase 4: proj + LN1 + transpose ----------------
            y2 = [vp.tile([128, C], F32, tag=f"v{t}", name=f"y2_{t}")
                  for t in range(TT)]
            y2T = [ycp.tile([128, N], F32R, tag=f"yc{c}", name=f"y2T{c}")
                   for c in range(CC)]
            with tc.tile_pool(name="wpj", bufs=1) as wpj, \
                 tc.tile_pool(name="atn", bufs=2) as atn, \
                 tc.tile_pool(name="ppj", bufs=2, space="PSUM") as ppj, \
                 tc.tile_pool(name="tpy", bufs=4, space="PSUM") as tpy:
                wp = [wpj.tile([128, C], F32R, tag=f"wp{c}", name=f"wp{c}")
                      for c in range(CC)]
                for c in range(CC):
                    nc.sync.dma_start(wp[c], proj_w[c * 128:(c + 1) * 128, :])
                for t in range(TT):
                    ps0 = ppj.tile([128, 512], F32, tag="a")
                    ps1 = ppj.tile([128, 512], F32, tag="b")
                    for c in range(CC):
                        st, sp = (c == 0), (c == CC - 1)
                        nc.tensor.matmul(ps0, ycat[c][:, bass.ts(t, 128)],
                                         wp[c][:, 0:512], start=st, stop=sp)
                        nc.tensor.matmul(ps1, ycat[c][:, bass.ts(t, 128)],
                                         wp[c][:, 512:1024], start=st, stop=sp)
                    at = atn.tile([128, C], F32, tag="at")
                    nc.vector.tensor_tensor(at[:, 0:512], ps0, pb_bc[:, 0:512],
                                            op=ALU.add)
                    nc.vector.tensor_tensor(at[:, 512:1024], ps1, pb_bc[:, 512:1024],
                                            op=ALU.add)
                    _ln_apply(nc, lnp, at, n1g_bc, n1b_bc, eps_t, y2[t], at)
                    for c in range(CC):
                        ps = tpy.tile([128, 128], F32, tag="t")
                        nc.tensor.transpose(ps, y2[t][:, bass.ts(c, 128)], idt)
                        nc.vector.tensor_copy(y2T[c][:, bass.ts(t, 128)], ps)

            consts_e_cm.__exit__(None, None, None)
            # ---------------- phase 5-7: MLP + LN2 + out (512-token slabs) ----
            with tc.tile_pool(name="hTp", bufs=1) as hTp, \
                 tc.tile_pool(name="w12", bufs=6) as w12, \
                 tc.tile_pool(name="h2t", bufs=3) as h2t, \
                 tc.tile_pool(name="h2p", bufs=1) as h2p, \
                 tc.tile_pool(name="fin", bufs=2) as fin, \
                 tc.tile_pool(name="pf1", bufs=3, space="PSUM") as pf1, \
                 tc.tile_pool(name="pf2", bufs=3, space="PSUM") as pf2, \
                 tc.tile_pool(name="tph", bufs=2, space="PSUM") as tph:
                for s in range(QT):
                    ssl = bass.ts(s, 512)
                    hT = [hTp.tile([128, 512], F32R, tag=f"h{hc}", name=f"h{hc}")
                          for hc in range(HC)]
                    h2 = [h2p.tile([128, C], F32, tag=f"h2_{i}", name=f"h2_{s}_{i}")
                          for i in range(4)]
                    # fc1 + gelu -> h^T (feature-major)
                    for hc in range(HC):
                        ph = pf1.tile([128, 512], F32, tag="a")
                        for c in range(CC):
                            w1 = w12.tile([128, 128], F32R, tag="w1")
                            nc.sync.dma_start(
                                w1, fc1_w[c * 128:(c + 1) * 128,
                                          hc * 128:(hc + 1) * 128])
                            nc.tensor.matmul(ph, w1, y2T[c][:, ssl],
                                             start=(c == 0), stop=(c == CC - 1))
                        nc.scalar.activation(hT[hc], ph, AF.Gelu,
                                             bias=fc1b[:, hc:hc + 1])
                    # fc2 -> h2^T chunks, transpose to token-major h2
                    for ct in range(CC):
                        pq = pf2.tile([128, 512], F32, tag="a")
                        for hc in range(HC):
                            w2 = w12.tile([128, 128], F32R, tag="w2")
                            nc.sync.dma_start(
                                w2, fc2_w[hc * 128:(hc + 1) * 128,
                                          ct * 128:(ct + 1) * 128])
                            nc.tensor.matmul(pq, w2, hT[hc],
                                             start=(hc == 0), stop=(hc == HC - 1))
                        h2T = h2t.tile([128, 512], F32, tag="h2T")
                        nc.scalar.activation(h2T, pq, AF.Identity,
                                             bias=fc2b[:, ct:ct + 1])
                        for i in range(4):
                            ps = tph.tile([128, 128], F32, tag="t")
                            nc.tensor.transpose(ps, h2T[:, bass.ts(i, 128)], idt)
                            nc.vector.tensor_copy(h2[i][:, bass.ts(ct, 128)], ps)
                    # LN2 + residual + store
                    for i in range(4):
                        t = s * 4 + i
                        ot = fin.tile([128, C], F32, tag="o")
                        _ln_apply(nc, lnp, h2[i], n2g_bc, n2b_bc, eps_t, ot, y2[t])
                        nc.sync.dma_start(out[t * 128:(t + 1) * 128, :], ot)

    nc.compile()
    return nc


_NC_CACHE = None


def kernel(**inputs):
    global _NC_CACHE
    if _NC_CACHE is None:
        _NC_CACHE = build()
    nc = _NC_CACHE

    wnames = ["qkv_w", "qkv_b", "proj_w", "proj_b", "n1_g", "n1_b",
              "fc1_w", "fc1_b", "fc2_w", "fc2_b", "n2_g", "n2_b"]
    shared = {k: np.ascontiguousarray(np.asarray(inputs[k], dtype=np.float32))
              for k in wnames}
    x = np.asarray(inputs["x"], dtype=np.float32)
    in_maps = [dict(shared, x=np.ascontiguousarray(x[b])) for b in range(B)]
    res = run_bass_kernel_spmd(nc, in_maps, list(range(B)))
    return np.stack([res.results[b]["out"] for b in range(B)]).astype(np.float32)
